# revision 12
# baseline (speedup 1.0000x reference)
"""AutoGNN-IDS Trainium2 kernel: 2x GATConv + 2x SAGEConv + fusion + 3 decoder heads.

Strategy: nodes are dst-sharded across 8 cores (6250/core, padded to 6272 =
49 blocks of 128). Edges live on the core owning their dst, sorted by
(dst block, src-range, dst), padded to static per-block tile counts shared
across cores. All segment reductions are one-hot matmuls on the PE (PSUM
accumulation per 128-node block); per-edge source-node features come from
SWDGE dma_gather out of replicated node tables; per-edge dst-node values come
from dma_gather out of per-core local tables (int16-safe local indices).
Softmax max-subtraction is skipped (mathematically identical, logits are
O(10) so fp32 exp cannot overflow). Self-loop terms are handled densely in
per-block epilogues. Layer-2 tables are AllGathered between phases.
"""
import math
import os
from contextlib import ExitStack

import numpy as np

N, E = 50000, 800000
NFD, EFD = 64, 16
GHD, GH, SHD = 32, 4, 64
EMB = GHD + SHD
NCORES = 8
M = N // NCORES            # 6250 real nodes per core
NB = 49                    # node blocks per core
MP = NB * 128              # 6272 padded nodes per core
NP = NCORES * MP           # 50176 padded global nodes
AB = 32768                 # int16 A/B table split
GMAX = 1024                # max num_idxs per dma_gather (HW ring limit)

_cache = {}


def _ceil(a, b):
    return -(-a // b)


def _wrap16(arr_l):
    """[L] int -> [128, L/16] wrapped int16 (replicated per 16-partition group)."""
    L = arr_l.shape[0]
    w = arr_l.reshape(L // 16, 16).T.astype(np.int16)   # [16, L/16]
    return np.tile(w, (8, 1))


def _lane_major(arr_l, width=None):
    """[L(,k)] -> [128, L/128(,k)] with lane l -> (l%128, l//128)."""
    L = arr_l.shape[0]
    rest = arr_l.shape[1:]
    return np.swapaxes(arr_l.reshape(L // 128, 128, *rest), 0, 1)


def _host_prep(x, edge_index, edge_attr, W):
    ei = np.asarray(edge_index)
    src0 = ei[0].astype(np.int64)
    dst0 = ei[1].astype(np.int64)
    ea = np.asarray(edge_attr, dtype=np.float32)

    cs = src0 // M
    g = (cs * MP + (src0 - cs * M)).astype(np.int64)    # padded global src id
    cd = dst0 // M
    ldst = (dst0 - cd * M).astype(np.int64)             # local dst id
    blk = ldst // 128
    rel = ldst % 128
    r = (g >= AB).astype(np.int64)

    deg = np.bincount(dst0, minlength=N).astype(np.float32)
    invdeg_full = 1.0 / np.maximum(deg, 1.0)

    # per (core, block, range) edge counts -> static tile structure
    key = (cd * NB + blk) * 2 + r
    cnt = np.bincount(key, minlength=NCORES * NB * 2).reshape(NCORES, NB, 2)
    TA = np.maximum(1, _ceil(cnt[:, :, 0].max(axis=0), 128))     # [NB] tiles
    TB = np.maximum(1, _ceil(cnt[:, :, 1].max(axis=0), 128))
    T = TA + TB
    toff = np.concatenate([[0], np.cumsum(T)])                   # tile offsets
    LT = int(toff[-1])
    L = LT * 128

    # lane assignment: edges sorted by (core, blk, r, ldst)
    order = np.lexsort((ldst, r, blk, cd))
    s_cd, s_blk, s_r = cd[order], blk[order], r[order]
    s_key = (s_cd * NB + s_blk) * 2 + s_r
    # rank within each (c,blk,r) group in sorted order
    grp_start_idx = np.concatenate([[0], np.cumsum(np.bincount(s_key, minlength=NCORES * NB * 2))])
    rank = np.arange(E) - grp_start_idx[s_key]
    seg_base = (toff[s_blk] + s_r * TA[s_blk]) * 128
    lane = seg_base + rank                                        # per sorted edge
    lane_of_edge = np.empty(E, dtype=np.int64)
    lane_of_edge[order] = lane + s_cd * L

    per_core = []
    for c in range(NCORES):
        sel = order[s_cd == c]
        lanes_c = lane[s_cd == c]
        gsrc = np.zeros(L, np.int64)
        gdst = np.zeros(L, np.int64)
        drel = -np.ones(L, np.float32)
        eal = np.zeros((L, EFD), np.float32)
        gsrc[lanes_c] = g[sel] - AB * r[sel]
        gdst[lanes_c] = ldst[sel]
        drel[lanes_c] = rel[sel].astype(np.float32)
        eal[lanes_c] = ea[sel]
        iv = np.ones(MP, np.float32)
        iv[:M] = invdeg_full[c * M:(c + 1) * M]
        per_core.append({
            "gsrc_w": _wrap16(gsrc),
            "gdst_w": _wrap16(gdst),
            "dstrel": np.ascontiguousarray(_lane_major(drel)),
            "ea_T": np.ascontiguousarray(eal.T),
            "invdeg": np.ascontiguousarray(iv.reshape(NB, 128).T),
            "xT_loc": None,  # filled below
        })

    # node features in padded layout
    x = np.asarray(x, dtype=np.float32)
    x_pad = np.zeros((NP, NFD), np.float32)
    for c in range(NCORES):
        x_pad[c * MP:c * MP + M] = x[c * M:(c + 1) * M]
    xT_pad = np.ascontiguousarray(x_pad.T)
    for c in range(NCORES):
        per_core[c]["xT_loc"] = np.ascontiguousarray(x_pad[c * MP:(c + 1) * MP].T)

    # weight preprocessing (pure reshapes/contractions of weights)
    W1, as1, ad1 = W["W1"], W["as1"], W["ad1"]
    W1a = np.stack([W1[:, h * GHD:(h + 1) * GHD] @ as1[h] for h in range(GH)], axis=1)
    W1b = np.stack([W1[:, h * GHD:(h + 1) * GHD] @ ad1[h] for h in range(GH)], axis=1)
    W1ext = np.concatenate([W1, W1a, W1b], axis=1).astype(np.float32)       # [64,136]
    we1 = np.stack([W["We1"][:, h * GHD:(h + 1) * GHD] @ W["ae1"][h] for h in range(GH)], axis=1)
    we2 = (W["We2"] @ W["ae2"][0])[:, None]
    weeff = np.concatenate([we1, we2], axis=1).astype(np.float32)           # [16,5]
    W2, as2, ad2 = W["W2"], W["as2"], W["ad2"]
    W2ext = np.concatenate([W2, (W2 @ as2[0])[:, None], (W2 @ ad2[0])[:, None]],
                           axis=1).astype(np.float32)                        # [128,34]
    b2row = np.zeros((1, 33), np.float32)
    b2row[0, 1:] = W["b2"]

    shared = {
        "x_pad": x_pad, "xT_pad": xT_pad,
        "W1ext": W1ext, "weeff": weeff, "W2ext": W2ext,
        "Wl1": W["Wl1"].astype(np.float32), "Wr1": W["Wr1"].astype(np.float32),
        "bl1": W["bl1"].reshape(1, -1).astype(np.float32),
        "Wl2": W["Wl2"].astype(np.float32), "Wr2": W["Wr2"].astype(np.float32),
        "bl2": W["bl2"].reshape(1, -1).astype(np.float32),
        "Wf": W["Wf"].astype(np.float32), "bf": W["bf"].reshape(1, -1).astype(np.float32),
        "Wd1": W["Wd1"].astype(np.float32), "bd1": W["bd1"].reshape(1, -1).astype(np.float32),
        "Wd2": W["Wd2"].astype(np.float32), "bd2": W["bd2"].reshape(1, -1).astype(np.float32),
        "Wa1": W["Wa1"].astype(np.float32), "ba1": W["ba1"].reshape(1, -1).astype(np.float32),
        "Wa2": W["Wa2"].astype(np.float32), "ba2": W["ba2"].reshape(1, -1).astype(np.float32),
        "b1row": W["b1"].reshape(1, -1).astype(np.float32),
        "b2row": b2row,
    }
    struct = {"TA": TA.tolist(), "TB": TB.tolist(), "T": T.tolist(),
              "toff": toff.tolist(), "LT": LT, "L": L,
              "lane_of_edge": lane_of_edge}
    return shared, per_core, struct


def _gather_chunks(nc, pool, out_tile, tab_ap, idx_dram, lane0, nlanes, tag,
                   elem, tile_off=0):
    """Issue <=GMAX-idx dma_gather chunks covering lanes [lane0, lane0+nlanes),
    writing output tiles starting at out_tile[:, tile_off, :]."""
    import concourse.mybir as mybir
    done = 0
    while done < nlanes:
        n = min(GMAX, nlanes - done)
        l0 = lane0 + done
        gi = pool.tile([128, n // 16], mybir.dt.int16, tag=tag)
        nc.sync.dma_start(gi[:], idx_dram[:, l0 // 16:(l0 + n) // 16])
        nc.gpsimd.dma_gather(
            out_ap=out_tile[:, tile_off + done // 128:tile_off + (done + n) // 128, :],
            in_ap=tab_ap, idxs_ap=gi[:],
            num_idxs=n, num_idxs_reg=n, elem_size=elem)
        done += n


def _build_program(struct, debug=False):
    import concourse.bacc as bacc
    import concourse.tile as tile
    from concourse import masks, mybir
    dt = mybir.dt
    Alu = mybir.AluOpType
    Act = mybir.ActivationFunctionType

    TA, TB, T, toff, LT, L = (struct["TA"], struct["TB"], struct["T"],
                              struct["toff"], struct["LT"], struct["L"])

    nc = bacc.Bacc("TRN2", target_bir_lowering=False, debug=False, num_devices=NCORES)

    def din(name, shape, d=dt.float32):
        return nc.dram_tensor(name, shape, d, kind="ExternalInput").ap()

    def dout(name, shape, d=dt.float32):
        return nc.dram_tensor(name, shape, d, kind="ExternalOutput").ap()

    # inputs
    x_pad = din("x_pad", [NP, NFD])
    xT_pad = din("xT_pad", [NFD, NP])
    xT_loc = din("xT_loc", [NFD, MP])
    ea_T = din("ea_T", [EFD, L])
    gsrc_w = din("gsrc_w", [128, L // 16], dt.int16)
    gdst_w = din("gdst_w", [128, L // 16], dt.int16)
    dstrel = din("dstrel", [128, LT])
    invdeg = din("invdeg", [128, NB])
    W1ext = din("W1ext", [NFD, 136])
    weeff = din("weeff", [EFD, 5])
    W2ext = din("W2ext", [128, 34])
    Wl1 = din("Wl1", [NFD, SHD]); Wr1 = din("Wr1", [NFD, SHD]); bl1 = din("bl1", [1, SHD])
    Wl2 = din("Wl2", [SHD, SHD]); Wr2 = din("Wr2", [SHD, SHD]); bl2 = din("bl2", [1, SHD])
    Wf = din("Wf", [EMB, EMB]); bf = din("bf", [1, EMB])
    Wd1 = din("Wd1", [EMB, GHD]); bd1 = din("bd1", [1, GHD])
    Wd2 = din("Wd2", [GHD, NFD]); bd2 = din("bd2", [1, NFD])
    Wa1 = din("Wa1", [EMB, GHD]); ba1 = din("ba1", [1, GHD])
    Wa2 = din("Wa2", [GHD, 1]); ba2 = din("ba2", [1, 1])
    b1row = din("b1row", [1, 128])
    b2row = din("b2row", [1, 33])

    # outputs (per-core shards)
    z_out = dout("z_out", [MP, EMB])
    scores_out = dout("scores_out", [MP, 1])
    feat_out = dout("feat_out", [MP, NFD])
    a2self_out = dout("a2self_out", [MP, 1])
    adj_out = dout("adj_out", [128, LT])
    a2e_out = dout("a2e_out", [128, LT])
    if debug:
        dbg_T1a = dout("dbg_T1a", [2048, 256])
        dbg_T2loc = dout("dbg_T2loc", [MP, 128])
        dbg_T3loc = dout("dbg_T3loc", [MP, 128])
        dbg_Tdst1 = dout("dbg_Tdst1", [MP, 64])
        dbg_Tdst2 = dout("dbg_Tdst2", [MP, 64])
        dbg_e2 = dout("dbg_e2", [128, LT])
        dbg_ev2 = dout("dbg_ev2", [128, LT])

    # internal DRAM
    T1a = nc.dram_tensor("T1a", [NP, 256], dt.float32).ap()
    Tdst1 = nc.dram_tensor("Tdst1", [MP, 64], dt.float32).ap()
    h1xloc = nc.dram_tensor("h1xloc", [MP, 128], dt.float32).ap()
    T2loc = nc.dram_tensor("T2loc", [MP, 128], dt.float32).ap()
    T2g = nc.dram_tensor("T2g", [NP, 128], dt.float32, addr_space="Shared").ap()
    Tdst2 = nc.dram_tensor("Tdst2", [MP, 64], dt.float32).ap()
    T3loc = nc.dram_tensor("T3loc", [MP, 128], dt.float32).ap()
    T3g = nc.dram_tensor("T3g", [NP, 128], dt.float32, addr_space="Shared").ap()
    sT_loc = nc.dram_tensor("sT_loc", [SHD, MP], dt.float32).ap()
    eatt2_st = nc.dram_tensor("eatt2_st", [128, LT], dt.float32).ap()
    expv2_st = nc.dram_tensor("expv2_st", [128, LT], dt.float32).ap()

    with tile.TileContext(nc) as tc:
        with ExitStack() as ctx:
            cpool = ctx.enter_context(tc.tile_pool(name="const", bufs=1))
            wpool = ctx.enter_context(tc.tile_pool(name="wts", bufs=1))
            p0 = ctx.enter_context(tc.tile_pool(name="p0", bufs=3))
            ps0 = ctx.enter_context(tc.tile_pool(name="ps0", bufs=2, space="PSUM"))
            gp = ctx.enter_context(tc.tile_pool(name="gp", bufs=2))
            bp = ctx.enter_context(tc.tile_pool(name="bp", bufs=2))
            ep = ctx.enter_context(tc.tile_pool(name="ep", bufs=2))
            segp = ctx.enter_context(tc.tile_pool(name="segp", bufs=1, space="PSUM"))
            psq = ctx.enter_context(tc.tile_pool(name="psq", bufs=3, space="PSUM"))

            # ---- constants ----
            ident = cpool.tile([128, 128], dt.float32)
            masks.make_identity(nc, ident[:])
            iota_i = cpool.tile([128, 128], dt.int32)
            nc.gpsimd.iota(iota_i[:], pattern=[[1, 128]], base=0, channel_multiplier=0)
            iota_f = cpool.tile([128, 128], dt.float32)
            nc.vector.tensor_copy(iota_f[:], iota_i[:])
            ones1 = cpool.tile([1, 128], dt.float32)
            nc.vector.memset(ones1[:], 1.0)

            def wtile(ap, p, f, tag):
                t = wpool.tile([p, f], dt.float32, tag=tag)
                nc.sync.dma_start(t[:], ap[:])
                return t

            W1e_sb = wtile(W1ext, NFD, 136, "w1e")
            weeff_sb = wtile(weeff, EFD, 5, "weeff")
            W2e_sb = wtile(W2ext, 128, 34, "w2e")
            Wl1_sb = wtile(Wl1, NFD, SHD, "wl1"); Wr1_sb = wtile(Wr1, NFD, SHD, "wr1")
            bl1_sb = wtile(bl1, 1, SHD, "bl1")
            Wl2_sb = wtile(Wl2, SHD, SHD, "wl2"); Wr2_sb = wtile(Wr2, SHD, SHD, "wr2")
            bl2_sb = wtile(bl2, 1, SHD, "bl2")
            Wf_sb = wtile(Wf, EMB, EMB, "wf"); bf_sb = wtile(bf, 1, EMB, "bf")
            Wd1_sb = wtile(Wd1, EMB, GHD, "wd1"); bd1_sb = wtile(bd1, 1, GHD, "bd1")
            Wd2_sb = wtile(Wd2, GHD, NFD, "wd2"); bd2_sb = wtile(bd2, 1, NFD, "bd2")
            Wa1_sb = wtile(Wa1, EMB, GHD, "wa1"); ba1_sb = wtile(ba1, 1, GHD, "ba1")
            Wa2_sb = wtile(Wa2, GHD, 1, "wa2"); ba2_sb = wtile(ba2, 1, 1, "ba2")
            b1_sb = wtile(b1row, 1, 128, "b1r")
            b2_sb = wtile(b2row, 1, 33, "b2r")

            # ---- P0: build T1a (full padded N) + local Tdst1/h1xloc ----
            for k in range(NP // 128):
                xt = p0.tile([NFD, 128], dt.float32, tag="xt")
                nc.sync.dma_start(xt[:], xT_pad[:, k * 128:(k + 1) * 128])
                ps = ps0.tile([128, 136], dt.float32, tag="p0ps")
                nc.tensor.matmul(ps[:], lhsT=xt[:], rhs=W1e_sb[:], start=True, stop=True)
                sb = p0.tile([128, 136], dt.float32, tag="p0sb")
                nc.vector.tensor_copy(sb[:], ps[:])
                nc.sync.dma_start(T1a[k * 128:(k + 1) * 128, 0:136], sb[:])
                nc.sync.dma_start(T1a[k * 128:(k + 1) * 128, 136:200],
                                  x_pad[k * 128:(k + 1) * 128, :])
            for k in range(NB):
                xt = p0.tile([NFD, 128], dt.float32, tag="xt")
                nc.sync.dma_start(xt[:], xT_loc[:, k * 128:(k + 1) * 128])
                ps = ps0.tile([128, 136], dt.float32, tag="p0ps")
                nc.tensor.matmul(ps[:], lhsT=xt[:], rhs=W1e_sb[:], start=True, stop=True)
                sb = p0.tile([128, 136], dt.float32, tag="p0sb")
                nc.vector.tensor_copy(sb[:], ps[:])
                nc.sync.dma_start(Tdst1[k * 128:(k + 1) * 128, 0:8], sb[:, 128:136])
                nc.sync.dma_start(h1xloc[k * 128:(k + 1) * 128, :], sb[:, 0:128])

            # ---- P1: GAT1 + SAGE1 edge phase ----
            for b in range(NB):
                Tb, TAb, TBb = T[b], TA[b], TB[b]
                lane0 = toff[b] * 128
                nl = Tb * 128

                eaT_sb = gp.tile([EFD, Tb * 128], dt.float32, tag="eaT")
                nc.sync.dma_start(eaT_sb[:], ea_T[:, lane0:lane0 + nl])
                dr = bp.tile([128, Tb], dt.float32, tag="dr")
                nc.sync.dma_start(dr[:], dstrel[:, toff[b]:toff[b] + Tb])
                iv = bp.tile([128, 1], dt.float32, tag="iv")
                nc.sync.dma_start(iv[:], invdeg[:, b:b + 1])
                td1 = bp.tile([128, 8], dt.float32, tag="td1")
                nc.sync.dma_start(td1[:], Tdst1[b * 128:(b + 1) * 128, 0:8])
                h1xl = bp.tile([128, 128], dt.float32, tag="h1xl")
                nc.sync.dma_start(h1xl[:], h1xloc[b * 128:(b + 1) * 128, :])

                G1 = gp.tile([128, Tb, 256], dt.float32, tag="G")
                _gather_chunks(nc, bp, G1, T1a[:AB, :], gsrc_w, lane0, TAb * 128,
                               "giA", 256)
                _gather_chunks(nc, bp, G1, T1a[AB:, :], gsrc_w, lane0 + TAb * 128,
                               TBb * 128, "giB", 256, tile_off=TAb)
                D1 = gp.tile([128, Tb, 64], dt.float32, tag="D")
                _gather_chunks(nc, bp, D1, Tdst1[:], gdst_w, lane0, nl, "giD", 64)

                oh_blk = gp.tile([128, Tb, 128], dt.float32, tag="oh")
                rhs_blk = bp.tile([128, Tb, 9], dt.float32, tag="rhs")
                logit = bp.tile([128, Tb, 4], dt.float32, tag="lgt")
                for t in range(Tb):
                    nc.vector.tensor_scalar(out=oh_blk[:, t, :], in0=iota_f[:],
                                            scalar1=dr[:, t:t + 1], scalar2=None,
                                            op0=Alu.is_equal)
                    ea_ps = psq.tile([128, 5], dt.float32, tag="q")
                    nc.tensor.matmul(ea_ps[:], lhsT=eaT_sb[:, t * 128:(t + 1) * 128],
                                     rhs=weeff_sb[:], start=True, stop=True)
                    nc.vector.tensor_tensor(out=logit[:, t, :], in0=G1[:, t, 128:132],
                                            in1=D1[:, t, 4:8], op=Alu.add)
                    nc.vector.tensor_tensor(out=logit[:, t, :], in0=logit[:, t, :],
                                            in1=ea_ps[:, 0:4], op=Alu.add)
                    nc.vector.tensor_copy(rhs_blk[:, t, 4:9], ea_ps[:, 0:5])
                # leaky relu + exp on whole block
                lt = bp.tile([128, Tb, 4], dt.float32, tag="lt")
                nc.vector.tensor_scalar_mul(lt[:], logit[:], 0.2)
                nc.vector.tensor_tensor(out=logit[:], in0=logit[:], in1=lt[:], op=Alu.max)
                nc.scalar.activation(rhs_blk[:, :, 0:4], logit[:], Act.Exp)
                # store eatt2 lanes for phase 2
                nc.sync.dma_start(eatt2_st[:, toff[b]:toff[b] + Tb], rhs_blk[:, :, 8])

                vals2 = bp.tile([128, Tb, 128], dt.float32, tag="v2")
                seg9 = segp.tile([128, 9], dt.float32, tag="seg9")
                segx = segp.tile([128, 64], dt.float32, tag="segx")
                segv = segp.tile([128, 128], dt.float32, tag="segv")
                for t in range(Tb):
                    nc.vector.tensor_tensor(
                        out=vals2[:, t, :].rearrange("p (h c) -> p h c", h=GH),
                        in0=G1[:, t, 0:128].rearrange("p (h c) -> p h c", h=GH),
                        in1=rhs_blk[:, t, 0:4].unsqueeze(2).broadcast_to([128, 4, GHD]),
                        op=Alu.mult)
                    nc.tensor.matmul(seg9[:], lhsT=oh_blk[:, t, :], rhs=rhs_blk[:, t, :],
                                     start=(t == 0), stop=(t == Tb - 1))
                    nc.tensor.matmul(segx[:], lhsT=oh_blk[:, t, :], rhs=G1[:, t, 136:200],
                                     start=(t == 0), stop=(t == Tb - 1))
                    nc.tensor.matmul(segv[:], lhsT=oh_blk[:, t, :], rhs=vals2[:, t, :],
                                     start=(t == 0), stop=False)
                nc.tensor.matmul(segv[:], lhsT=ones1[:], rhs=b1_sb[:], start=False, stop=True)

                # ---- P1 epilogue ----
                eself = ep.tile([128, 5], dt.float32, tag="eself")
                nc.vector.tensor_scalar_mul(eself[:], seg9[:, 4:9], iv[:, 0:1])
                lgs = ep.tile([128, 4], dt.float32, tag="lgs")
                nc.vector.tensor_tensor(out=lgs[:], in0=td1[:, 4:8], in1=td1[:, 0:4], op=Alu.add)
                nc.vector.tensor_tensor(out=lgs[:], in0=lgs[:], in1=eself[:, 0:4], op=Alu.add)
                lgs2 = ep.tile([128, 4], dt.float32, tag="lgs2")
                nc.vector.tensor_scalar_mul(lgs2[:], lgs[:], 0.2)
                nc.vector.tensor_tensor(out=lgs[:], in0=lgs[:], in1=lgs2[:], op=Alu.max)
                evs = ep.tile([128, 4], dt.float32, tag="evs")
                nc.scalar.activation(evs[:], lgs[:], Act.Exp)
                den = ep.tile([128, 4], dt.float32, tag="den")
                nc.vector.tensor_tensor(out=den[:], in0=seg9[:, 0:4], in1=evs[:], op=Alu.add)
                ivd = ep.tile([128, 4], dt.float32, tag="ivd")
                nc.vector.reciprocal(ivd[:], den[:])
                out1 = ep.tile([128, 4, GHD], dt.float32, tag="out1")
                nc.vector.tensor_tensor(
                    out=out1[:], in0=h1xl[:].rearrange("p (h c) -> p h c", h=GH),
                    in1=evs[:].unsqueeze(2).broadcast_to([128, 4, GHD]), op=Alu.mult)
                nc.vector.tensor_tensor(out=out1[:], in0=out1[:],
                                        in1=segv[:].rearrange("p (h c) -> p h c", h=GH),
                                        op=Alu.add)
                nc.vector.tensor_tensor(
                    out=out1[:], in0=out1[:],
                    in1=ivd[:].unsqueeze(2).broadcast_to([128, 4, GHD]), op=Alu.mult)
                # elu -> h1
                h1 = ep.tile([128, 128], dt.float32, tag="h1")
                o1f = out1[:].rearrange("p h c -> p (h c)")
                nc.vector.tensor_scalar_min(h1[:], o1f, 0.0)
                nc.scalar.activation(h1[:], h1[:], Act.Exp)
                nc.vector.tensor_scalar_add(h1[:], h1[:], -1.0)
                nc.vector.tensor_tensor(out=h1[:], in0=h1[:], in1=o1f, op=Alu.max)
                # h2/asrc2/adst2
                h1T_ps = psq.tile([128, 128], dt.float32, tag="q")
                nc.tensor.transpose(h1T_ps[:], h1[:], ident[:])
                h1T = ep.tile([128, 128], dt.float32, tag="h1Ts")
                nc.vector.tensor_copy(h1T[:], h1T_ps[:])
                h2_ps = psq.tile([128, 34], dt.float32, tag="q")
                nc.tensor.matmul(h2_ps[:], lhsT=h1T[:], rhs=W2e_sb[:], start=True, stop=True)
                h2sb = ep.tile([128, 34], dt.float32, tag="h2sb")
                nc.vector.tensor_copy(h2sb[:], h2_ps[:])
                nc.sync.dma_start(T2loc[b * 128:(b + 1) * 128, 0:34], h2sb[:])
                nc.sync.dma_start(Tdst2[b * 128:(b + 1) * 128, 0:1], h2sb[:, 33:34])
                nc.sync.dma_start(Tdst2[b * 128:(b + 1) * 128, 1:2], eself[:, 4:5])
                # SAGE1
                aggr = ep.tile([128, 64], dt.float32, tag="aggr")
                nc.vector.tensor_scalar_mul(aggr[:], segx[:], iv[:, 0:1])
                agT_ps = psq.tile([64, 128], dt.float32, tag="q")
                nc.tensor.transpose(agT_ps[:], aggr[:], ident[:])
                agT = ep.tile([64, 128], dt.float32, tag="agTs")
                nc.vector.tensor_copy(agT[:], agT_ps[:])
                xt = p0.tile([NFD, 128], dt.float32, tag="xt")
                nc.sync.dma_start(xt[:], xT_loc[:, b * 128:(b + 1) * 128])
                s_ps = psq.tile([128, 64], dt.float32, tag="q")
                nc.tensor.matmul(s_ps[:], lhsT=agT[:], rhs=Wl1_sb[:], start=True, stop=False)
                nc.tensor.matmul(s_ps[:], lhsT=xt[:], rhs=Wr1_sb[:], start=False, stop=False)
                nc.tensor.matmul(s_ps[:], lhsT=ones1[:], rhs=bl1_sb[:], start=False, stop=True)
                s_sb = ep.tile([128, 64], dt.float32, tag="ssb")
                nc.scalar.activation(s_sb[:], s_ps[:], Act.Relu)
                nc.sync.dma_start(T2loc[b * 128:(b + 1) * 128, 34:98], s_sb[:])
                sT_ps = psq.tile([64, 128], dt.float32, tag="q")
                nc.tensor.transpose(sT_ps[:], s_sb[:], ident[:])
                sTsb = ep.tile([64, 128], dt.float32, tag="sTsb")
                nc.vector.tensor_copy(sTsb[:], sT_ps[:])
                nc.sync.dma_start(sT_loc[:, b * 128:(b + 1) * 128], sTsb[:])

            # ---- AllGather T2 ----
            nc.gpsimd.collective_compute(
                "AllGather", Alu.bypass, replica_groups=[list(range(NCORES))],
                ins=[T2loc[:]], outs=[T2g[:]])

            # ---- P2: GAT2 + SAGE2 edge phase ----
            for b in range(NB):
                Tb, TAb, TBb = T[b], TA[b], TB[b]
                lane0 = toff[b] * 128
                nl = Tb * 128

                dr = bp.tile([128, Tb], dt.float32, tag="dr")
                nc.sync.dma_start(dr[:], dstrel[:, toff[b]:toff[b] + Tb])
                iv = bp.tile([128, 1], dt.float32, tag="iv")
                nc.sync.dma_start(iv[:], invdeg[:, b:b + 1])
                e2l = bp.tile([128, Tb], dt.float32, tag="e2l")
                nc.sync.dma_start(e2l[:], eatt2_st[:, toff[b]:toff[b] + Tb])

                G2 = gp.tile([128, Tb, 128], dt.float32, tag="G")
                _gather_chunks(nc, bp, G2, T2g[:AB, :], gsrc_w, lane0, TAb * 128,
                               "giA", 128)
                _gather_chunks(nc, bp, G2, T2g[AB:, :], gsrc_w, lane0 + TAb * 128,
                               TBb * 128, "giB", 128, tile_off=TAb)
                D2 = gp.tile([128, Tb, 64], dt.float32, tag="D")
                _gather_chunks(nc, bp, D2, Tdst2[:], gdst_w, lane0, nl, "giD", 64)

                oh_blk = gp.tile([128, Tb, 128], dt.float32, tag="oh")
                lg2 = bp.tile([128, Tb], dt.float32, tag="lg2")
                rhs2 = bp.tile([128, Tb, 33], dt.float32, tag="rhs2")
                for t in range(Tb):
                    nc.vector.tensor_scalar(out=oh_blk[:, t, :], in0=iota_f[:],
                                            scalar1=dr[:, t:t + 1], scalar2=None,
                                            op0=Alu.is_equal)
                    nc.vector.tensor_tensor(out=lg2[:, t:t + 1], in0=G2[:, t, 32:33],
                                            in1=D2[:, t, 0:1], op=Alu.add)
                nc.vector.tensor_tensor(out=lg2[:], in0=lg2[:], in1=e2l[:], op=Alu.add)
                lt2 = bp.tile([128, Tb], dt.float32, tag="lt2")
                nc.vector.tensor_scalar_mul(lt2[:], lg2[:], 0.2)
                nc.vector.tensor_tensor(out=lg2[:], in0=lg2[:], in1=lt2[:], op=Alu.max)
                ev2 = bp.tile([128, Tb], dt.float32, tag="ev2")
                nc.scalar.activation(ev2[:], lg2[:], Act.Exp)
                nc.sync.dma_start(expv2_st[:, toff[b]:toff[b] + Tb], ev2[:])
                nc.vector.tensor_copy(rhs2[:, :, 0:1], ev2[:].unsqueeze(2))
                segA = segp.tile([128, 33], dt.float32, tag="seg9")
                segS = segp.tile([128, 64], dt.float32, tag="segx")
                for t in range(Tb):
                    nc.vector.tensor_scalar_mul(rhs2[:, t, 1:33], G2[:, t, 0:32],
                                                ev2[:, t:t + 1])
                    nc.tensor.matmul(segA[:], lhsT=oh_blk[:, t, :], rhs=rhs2[:, t, :],
                                     start=(t == 0), stop=False)
                    nc.tensor.matmul(segS[:], lhsT=oh_blk[:, t, :], rhs=G2[:, t, 34:98],
                                     start=(t == 0), stop=(t == Tb - 1))
                nc.tensor.matmul(segA[:], lhsT=ones1[:], rhs=b2_sb[:], start=False, stop=True)

                # ---- P2 epilogue ----
                t2l = ep.tile([128, 33], dt.float32, tag="t2l")
                nc.sync.dma_start(t2l[:], T2loc[b * 128:(b + 1) * 128, 0:33])
                td2 = ep.tile([128, 2], dt.float32, tag="td2")
                nc.sync.dma_start(td2[:], Tdst2[b * 128:(b + 1) * 128, 0:2])
                lgs = ep.tile([128, 1], dt.float32, tag="lgs")
                nc.vector.tensor_tensor(out=lgs[:], in0=t2l[:, 32:33], in1=td2[:, 0:1], op=Alu.add)
                nc.vector.tensor_tensor(out=lgs[:], in0=lgs[:], in1=td2[:, 1:2], op=Alu.add)
                lgs2 = ep.tile([128, 1], dt.float32, tag="lgs2")
                nc.vector.tensor_scalar_mul(lgs2[:], lgs[:], 0.2)
                nc.vector.tensor_tensor(out=lgs[:], in0=lgs[:], in1=lgs2[:], op=Alu.max)
                ev2s = ep.tile([128, 1], dt.float32, tag="evs")
                nc.scalar.activation(ev2s[:], lgs[:], Act.Exp)
                den = ep.tile([128, 1], dt.float32, tag="den")
                nc.vector.tensor_tensor(out=den[:], in0=segA[:, 0:1], in1=ev2s[:], op=Alu.add)
                ivd2 = ep.tile([128, 1], dt.float32, tag="ivd")
                nc.vector.reciprocal(ivd2[:], den[:])
                a2s = ep.tile([128, 1], dt.float32, tag="a2s")
                nc.vector.tensor_tensor(out=a2s[:], in0=ev2s[:], in1=ivd2[:], op=Alu.mult)
                nc.sync.dma_start(a2self_out[b * 128:(b + 1) * 128, :], a2s[:])
                out2 = ep.tile([128, GHD], dt.float32, tag="out2")
                nc.vector.tensor_scalar_mul(out2[:], t2l[:, 0:32], ev2s[:, 0:1])
                nc.vector.tensor_tensor(out=out2[:], in0=out2[:], in1=segA[:, 1:33], op=Alu.add)
                nc.vector.tensor_scalar_mul(out2[:], out2[:], ivd2[:, 0:1])
                zg = ep.tile([128, GHD], dt.float32, tag="zg")
                nc.vector.tensor_scalar_min(zg[:], out2[:], 0.0)
                nc.scalar.activation(zg[:], zg[:], Act.Exp)
                nc.vector.tensor_scalar_add(zg[:], zg[:], -1.0)
                nc.vector.tensor_tensor(out=zg[:], in0=zg[:], in1=out2[:], op=Alu.max)
                # SAGE2
                aggr = ep.tile([128, 64], dt.float32, tag="aggr")
                nc.vector.tensor_scalar_mul(aggr[:], segS[:], iv[:, 0:1])
                agT_ps = psq.tile([64, 128], dt.float32, tag="q")
                nc.tensor.transpose(agT_ps[:], aggr[:], ident[:])
                agT = ep.tile([64, 128], dt.float32, tag="agTs")
                nc.vector.tensor_copy(agT[:], agT_ps[:])
                sT = ep.tile([64, 128], dt.float32, tag="sTl")
                nc.sync.dma_start(sT[:], sT_loc[:, b * 128:(b + 1) * 128])
                s2_ps = psq.tile([128, 64], dt.float32, tag="q")
                nc.tensor.matmul(s2_ps[:], lhsT=agT[:], rhs=Wl2_sb[:], start=True, stop=False)
                nc.tensor.matmul(s2_ps[:], lhsT=sT[:], rhs=Wr2_sb[:], start=False, stop=False)
                nc.tensor.matmul(s2_ps[:], lhsT=ones1[:], rhs=bl2_sb[:], start=False, stop=True)
                zcat = ep.tile([128, EMB], dt.float32, tag="zcat")
                nc.vector.tensor_copy(zcat[:, 0:GHD], zg[:])
                nc.scalar.activation(zcat[:, GHD:EMB], s2_ps[:], Act.Relu)
                zcT_ps = psq.tile([EMB, 128], dt.float32, tag="q")
                nc.tensor.transpose(zcT_ps[:], zcat[:], ident[:])
                zcT = ep.tile([EMB, 128], dt.float32, tag="zcTs")
                nc.vector.tensor_copy(zcT[:], zcT_ps[:])
                z_ps = psq.tile([128, EMB], dt.float32, tag="q")
                nc.tensor.matmul(z_ps[:], lhsT=zcT[:], rhs=Wf_sb[:], start=True, stop=False)
                nc.tensor.matmul(z_ps[:], lhsT=ones1[:], rhs=bf_sb[:], start=False, stop=True)
                z_sb = ep.tile([128, EMB], dt.float32, tag="zsb")
                nc.scalar.activation(z_sb[:], z_ps[:], Act.Relu)
                nc.sync.dma_start(z_out[b * 128:(b + 1) * 128, :], z_sb[:])
                nc.sync.dma_start(T3loc[b * 128:(b + 1) * 128, 0:96], z_sb[:])
                nc.sync.dma_start(T3loc[b * 128:(b + 1) * 128, 96:97], ivd2[:])
                # decoders
                zT_ps = psq.tile([EMB, 128], dt.float32, tag="q")
                nc.tensor.transpose(zT_ps[:], z_sb[:], ident[:])
                zT = ep.tile([EMB, 128], dt.float32, tag="zTs")
                nc.vector.tensor_copy(zT[:], zT_ps[:])
                d1_ps = psq.tile([128, GHD], dt.float32, tag="q")
                nc.tensor.matmul(d1_ps[:], lhsT=zT[:], rhs=Wd1_sb[:], start=True, stop=False)
                nc.tensor.matmul(d1_ps[:], lhsT=ones1[:], rhs=bd1_sb[:], start=False, stop=True)
                t1 = ep.tile([128, GHD], dt.float32, tag="t1")
                nc.scalar.activation(t1[:], d1_ps[:], Act.Relu)
                t1T_ps = psq.tile([GHD, 128], dt.float32, tag="q")
                nc.tensor.transpose(t1T_ps[:], t1[:], ident[:])
                t1T = ep.tile([GHD, 128], dt.float32, tag="t1Ts")
                nc.vector.tensor_copy(t1T[:], t1T_ps[:])
                d2_ps = psq.tile([128, NFD], dt.float32, tag="q")
                nc.tensor.matmul(d2_ps[:], lhsT=t1T[:], rhs=Wd2_sb[:], start=True, stop=False)
                nc.tensor.matmul(d2_ps[:], lhsT=ones1[:], rhs=bd2_sb[:], start=False, stop=True)
                d2sb = ep.tile([128, NFD], dt.float32, tag="d2sb")
                nc.vector.tensor_copy(d2sb[:], d2_ps[:])
                nc.sync.dma_start(feat_out[b * 128:(b + 1) * 128, :], d2sb[:])
                a1_ps = psq.tile([128, GHD], dt.float32, tag="q")
                nc.tensor.matmul(a1_ps[:], lhsT=zT[:], rhs=Wa1_sb[:], start=True, stop=False)
                nc.tensor.matmul(a1_ps[:], lhsT=ones1[:], rhs=ba1_sb[:], start=False, stop=True)
                u = ep.tile([128, GHD], dt.float32, tag="u")
                nc.scalar.activation(u[:], a1_ps[:], Act.Relu)
                uT_ps = psq.tile([GHD, 128], dt.float32, tag="q")
                nc.tensor.transpose(uT_ps[:], u[:], ident[:])
                uT = ep.tile([GHD, 128], dt.float32, tag="uTs")
                nc.vector.tensor_copy(uT[:], uT_ps[:])
                a2_ps = psq.tile([128, 1], dt.float32, tag="q")
                nc.tensor.matmul(a2_ps[:], lhsT=uT[:], rhs=Wa2_sb[:], start=True, stop=False)
                nc.tensor.matmul(a2_ps[:], lhsT=ones1[:], rhs=ba2_sb[:], start=False, stop=True)
                sc = ep.tile([128, 1], dt.float32, tag="sc")
                nc.scalar.activation(sc[:], a2_ps[:], Act.Sigmoid)
                nc.sync.dma_start(scores_out[b * 128:(b + 1) * 128, :], sc[:])

            if debug:
                nc.sync.dma_start(dbg_T2loc[:], T2loc[:])
                nc.sync.dma_start(dbg_T1a[:], T1a[0:2048, :])
                nc.sync.dma_start(dbg_Tdst1[:], Tdst1[:])
                nc.sync.dma_start(dbg_Tdst2[:], Tdst2[:])
                nc.sync.dma_start(dbg_e2[:], eatt2_st[:])
                nc.sync.dma_start(dbg_ev2[:], expv2_st[:])
                nc.sync.dma_start(dbg_T3loc[:], T3loc[:])

            # ---- AllGather T3 ----
            nc.gpsimd.collective_compute(
                "AllGather", Alu.bypass, replica_groups=[list(range(NCORES))],
                ins=[T3loc[:]], outs=[T3g[:]])

            # ---- P3: adj_recon + alpha2 edge phase ----
            for b in range(NB):
                Tb, TAb, TBb = T[b], TA[b], TB[b]
                lane0 = toff[b] * 128
                nl = Tb * 128
                G3 = gp.tile([128, Tb, 128], dt.float32, tag="G")
                _gather_chunks(nc, bp, G3, T3g[:AB, :], gsrc_w, lane0, TAb * 128,
                               "giA", 128)
                _gather_chunks(nc, bp, G3, T3g[AB:, :], gsrc_w, lane0 + TAb * 128,
                               TBb * 128, "giB", 128, tile_off=TAb)
                D3 = gp.tile([128, Tb, 128], dt.float32, tag="D")
                _gather_chunks(nc, bp, D3, T3loc[:], gdst_w, lane0, nl, "giD", 128)

                ev2 = bp.tile([128, Tb], dt.float32, tag="ev2")
                nc.sync.dma_start(ev2[:], expv2_st[:, toff[b]:toff[b] + Tb])
                prod = bp.tile([128, Tb, 96], dt.float32, tag="v2")
                nc.vector.tensor_tensor(out=prod[:], in0=G3[:, :, 0:96],
                                        in1=D3[:, :, 0:96], op=Alu.mult)
                adjl = bp.tile([128, Tb], dt.float32, tag="adjl")
                nc.vector.tensor_reduce(adjl[:], prod[:], axis=mybir.AxisListType.X,
                                        op=Alu.add)
                adjs = bp.tile([128, Tb], dt.float32, tag="adjs")
                nc.scalar.activation(adjs[:], adjl[:], Act.Sigmoid)
                nc.sync.dma_start(adj_out[:, toff[b]:toff[b] + Tb], adjs[:])
                a2e = bp.tile([128, Tb], dt.float32, tag="a2e")
                nc.vector.tensor_tensor(out=a2e[:], in0=ev2[:], in1=D3[:, :, 96],
                                        op=Alu.mult)
                nc.sync.dma_start(a2e_out[:, toff[b]:toff[b] + Tb], a2e[:])

    nc.compile()
    return nc


def _get_program(struct, debug=False):
    key = (tuple(struct["T"]), tuple(struct["TA"]), debug)
    if key not in _cache:
        _cache[key] = _build_program(struct, debug=debug)
    return _cache[key]


def kernel(**inputs):
    return _kernel_impl(inputs, debug=False)[0]


def _kernel_impl(inputs, debug=False):
    from concourse.bass_utils import run_bass_kernel_spmd

    x = np.asarray(inputs["x"], dtype=np.float32)
    W = {k: np.asarray(v, dtype=np.float32) for k, v in inputs.items()
         if k not in ("x", "edge_index", "edge_attr")}
    shared, per_core, struct = _host_prep(x, inputs["edge_index"],
                                          inputs["edge_attr"], W)
    nc = _get_program(struct, debug=debug)

    in_maps = []
    for c in range(NCORES):
        m = dict(shared)
        m.update(per_core[c])
        in_maps.append(m)
    res = run_bass_kernel_spmd(nc, in_maps, list(range(NCORES)))

    LT = struct["LT"]
    L = struct["L"]
    lane_of_edge = struct["lane_of_edge"]

    z = np.concatenate([res.results[c]["z_out"][:M] for c in range(NCORES)], axis=0)
    scores = np.concatenate([res.results[c]["scores_out"][:M, 0] for c in range(NCORES)])
    feat = np.concatenate([res.results[c]["feat_out"][:M] for c in range(NCORES)], axis=0)
    a2self = np.concatenate([res.results[c]["a2self_out"][:M, 0] for c in range(NCORES)])
    adj_flat = np.concatenate([res.results[c]["adj_out"].T.reshape(-1) for c in range(NCORES)])
    a2e_flat = np.concatenate([res.results[c]["a2e_out"].T.reshape(-1) for c in range(NCORES)])

    adj_recon = adj_flat[lane_of_edge].astype(np.float32)
    alpha2 = np.concatenate([a2e_flat[lane_of_edge], a2self])[:, None].astype(np.float32)
    return ((z.astype(np.float32), scores.astype(np.float32), adj_recon,
             feat.astype(np.float32), alpha2), res, struct)
